# revision 16
# baseline (speedup 1.0000x reference)
"""GQA causal attention (B=2, S=2048, HID=2048, H=32, HKV=8, D=128) on 8 TRN2
NeuronCores.

Sharding: tensor-parallel over heads for QKV+attention (core c owns kv head c
and q heads 4c..4c+3), then an AllToAll switches to sequence-parallel for
o_proj (core c computes the full hidden dim for global s-chunk c). The A2A
moves 8x less data than an AllGather and needs no per-core dynamic slicing.
It is split into two collectives (head pairs) so comm overlaps attention
compute of the remaining heads and the first half of o_proj.

Device pipeline (bf16 compute, fp32 PSUM accumulation):
  1. Feature-major projections: Q^T/K^T/V^T = W^T h^T, h^T streamed on the
     sync queue; all small constants stream on the scalar queue so they never
     stall the h^T pipeline.
  2. RoPE as  x*cos_dup + swap_halves(x)*sin_signed  - the rotate-half is a
     pure partition swap done by idle gpsimd SWDGE DMAs (the sign lives in the
     host-prepared sin table); cross-partition DVE ops are illegal.
  3. Transposed flash attention: S^T[k,q] = K^T_chunk.T @ Q^T chunk. Score
     tiles are computed in PAIRS into a 2-bank [128,1024] fp32 PSUM tile so
     one ScalarE exp covers two tiles (the ACT per-op overhead is ~352
     cycles; pairing keeps ACT off the critical path). Causal 0/1 masks for
     the two diagonal pairs are host-packed ([tril512|tril384|tril256|
     tril128]) so each pair needs one DVE mul. The denominator uses an
     all-ones [128,128] stationary (lands pre-broadcast across partitions);
     diagonal-pair P tiles are pre-summed on the DVE so the whole qc needs
     only ~3 denominator matmuls instead of one per tile.
  4. Two AllToAlls (heads 0-1, then 2-3) exchange attn-out^T blocks.
  5. o_proj: out^T[hid, my_s_chunk] accumulated over all 32 feature tiles.
     Wo is host-relaid-out to [128, 32*2048] so each weight tile is one
     contiguous DMA; all pass-0 tiles prefetch on the idle gpsimd queue
     DURING attention, so the PE flows from attention straight into o_proj
     without an idle window (which would re-throttle the PE clock).
Host reassembles the 8 sequence chunks and transposes back.
"""

import os

import numpy as np
import ml_dtypes

from concourse import bacc, mybir
import concourse.tile as tile
from concourse.bass_utils import run_bass_kernel_spmd

N_CORES = 8
B, S, HID = 2, 2048, 2048
H, HKV, D = 32, 8, 128
QH = H // HKV          # q heads per core
SG = B * S             # 4096 global sequence
NSC = SG // 512        # 8 s-chunks of 512
NKT = HID // 128       # 16 hid k-tiles

BF = mybir.dt.bfloat16
F32 = mybir.dt.float32
AF = mybir.ActivationFunctionType

_CACHE = {}
LAST_EXEC_NS = None


def _build():
    nc = bacc.Bacc("TRN2", num_devices=N_CORES)

    hT_e = nc.declare_dram_parameter("hT", [HID, SG], BF, isOutput=False)
    wq_e = nc.declare_dram_parameter("wq", [HID, QH * D], BF, isOutput=False)
    wk_e = nc.declare_dram_parameter("wk", [HID, D], BF, isOutput=False)
    wv_e = nc.declare_dram_parameter("wv", [HID, D], BF, isOutput=False)
    # Wo host-relaid-out: [p, (half, hid_t, ft, c)] so each [128, 2048] slice
    # is one o_proj weight tile, contiguous per partition.
    wo_e = nc.declare_dram_parameter("wo", [128, 32 * 16 * 128], BF, isOutput=False)
    cd_e = nc.declare_dram_parameter("c_dup", [D, SG], BF, isOutput=False)
    sd_e = nc.declare_dram_parameter("s_dup", [D, SG], BF, isOutput=False)
    # packed diagonal-pair masks: [tril512 | tril384 | tril256 | tril128]
    dm_e = nc.declare_dram_parameter("dmask", [128, 1280], BF, isOutput=False)
    outT_e = nc.declare_dram_parameter("outT", [HID, 512], F32, isOutput=True)

    with tile.TileContext(nc) as tc:
        with (
            tc.tile_pool(name="cst", bufs=1) as cst,
            tc.tile_pool(name="sb", bufs=2) as sb,
            tc.tile_pool(name="ps", bufs=2, space="PSUM") as ps,
            tc.tile_pool(name="dram", bufs=1, space="DRAM") as dram,
        ):
            dmask = cst.tile([128, 1280], BF, tag="dmask")
            ones_mat = cst.tile([128, 128], BF, tag="ones_mat")
            nc.gpsimd.memset(ones_mat[:], 1.0)

            qr = cst.tile([128, QH * SG], BF, tag="qr")
            kr = cst.tile([128, SG], BF, tag="kr")
            v_seq = cst.tile([128, SG], BF, tag="v_seq")

            # A2A bounce buffers. Collective 0 carries heads {0,1} (shard j =
            # 256 rows), collectives 1/2 carry head 2 / head 3 (shard j = 128
            # rows) so the last collective is small and lands well before the
            # o_proj second pass needs it.
            a2a_rows = (256, 128, 128)
            a2a_in = [
                dram.tile([8 * r, 512], BF, name=f"a2ain{i}", tag=f"a2ain{i}")
                for i, r in enumerate(a2a_rows)
            ]
            a2a_out = [
                dram.tile([8 * r, 512], BF, name=f"a2aout{i}", tag=f"a2aout{i}")
                for i, r in enumerate(a2a_rows)
            ]

            # ---- phase 1: projections + rope + V transpose ----
            with tc.tile_pool(name="p1", bufs=1) as p1, \
                 tc.tile_pool(name="htp", bufs=3) as htp:
                # piece-wise loads on the sync queue: per-kt pieces give
                # per-region deps, so the first matmul starts after ~256KB.
                # Everything else rides the scalar queue so the sync queue is
                # a pure wq/hT stream.
                wq_sb = p1.tile([128, NKT, QH * D], BF, tag="wq_sb")
                ht0 = htp.tile([128, NKT, 512], BF, tag="ht")
                c_d = p1.tile([D, SG], BF, tag="c_d")
                s_d = p1.tile([D, SG], BF, tag="s_d")
                # coarse 4-kt pieces: the Sync engine pays ~0.7us issue time
                # per DMA instruction, so fewer+bigger beats 32 small pieces.
                for kq in range(4):
                    nc.sync.dma_start(
                        wq_sb[:, 4 * kq : 4 * kq + 4, :],
                        wq_e[4 * kq * 128 : (4 * kq + 4) * 128, :].rearrange(
                            "(kt p) f -> p kt f", p=128
                        ),
                    )
                    nc.sync.dma_start(
                        ht0[:, 4 * kq : 4 * kq + 4, :],
                        hT_e[
                            4 * kq * 128 : (4 * kq + 4) * 128, 0:512
                        ].rearrange("(kt p) s -> p kt s", p=128),
                    )
                    if kq == 0:
                        # rope constants for the first chunk, off-queue
                        nc.scalar.dma_start(c_d[:, 0:512], cd_e[:, 0:512])
                        nc.scalar.dma_start(s_d[:, 0:512], sd_e[:, 0:512])
                wk_sb = p1.tile([128, NKT, D], BF, tag="wk_sb")
                nc.scalar.dma_start(
                    wk_sb[:], wk_e[:].rearrange("(kt p) f -> p kt f", p=128)
                )
                wv_sb = p1.tile([128, NKT, D], BF, tag="wv_sb")
                nc.scalar.dma_start(
                    wv_sb[:], wv_e[:].rearrange("(kt p) f -> p kt f", p=128)
                )
                nc.scalar.dma_start(dmask[:], dm_e[:])
                for sc in range(1, NSC):
                    nc.scalar.dma_start(
                        c_d[:, sc * 512 : (sc + 1) * 512],
                        cd_e[:, sc * 512 : (sc + 1) * 512],
                    )
                    nc.scalar.dma_start(
                        s_d[:, sc * 512 : (sc + 1) * 512],
                        sd_e[:, sc * 512 : (sc + 1) * 512],
                    )

                # rope/V-transpose for tile i are emitted AFTER projection
                # chain i+1 so their PE ops never wait on the ACT evacuation.
                def finish_tile(sc, ft, xb):
                    if ft < QH + 1:  # rope for q heads and k
                        # rotate-half = partition swap via idle gpsimd SWDGE
                        # (sin table sign-folded on host)
                        sh = p1.tile([128, 512], BF, tag="sh", bufs=3)
                        nc.gpsimd.dma_start(sh[0:64, :], xb[64:128, :])
                        nc.gpsimd.dma_start(sh[64:128, :], xb[0:64, :])
                        if ft < QH:
                            dest = qr[
                                :, ft * SG + sc * 512 : ft * SG + sc * 512 + 512
                            ]
                        else:
                            dest = kr[:, sc * 512 : sc * 512 + 512]
                        cs = c_d[:, sc * 512 : (sc + 1) * 512]
                        ss = s_d[:, sc * 512 : (sc + 1) * 512]
                        nc.vector.tensor_mul(dest, xb[:], cs)
                        rtmp = p1.tile([128, 512], BF, tag="rtmp", bufs=2)
                        nc.vector.tensor_mul(rtmp[:], sh[:], ss)
                        nc.vector.tensor_add(dest, dest, rtmp[:])
                    else:  # v: transpose to seq-major via the DMA xbar —
                        # keeps the 32 transposes off the TensorE FIFO.
                        for j in range(4):
                            g = sc * 4 + j
                            nc.scalar.dma_start(
                                v_seq[:, g * 128 : (g + 1) * 128],
                                xb[:, j * 128 : (j + 1) * 128],
                                transpose=True,
                            )

                with nc.named_scope("proj"):
                    pending = None
                    for sc in range(NSC):
                        if sc == 0:
                            ht = ht0
                        else:
                            ht = htp.tile([128, NKT, 512], BF, tag="ht")
                            for kq in range(4):  # 4 coarser pieces
                                nc.sync.dma_start(
                                    ht[:, 4 * kq : 4 * kq + 4, :],
                                    hT_e[
                                        4 * kq * 128 : (4 * kq + 4) * 128,
                                        sc * 512 : (sc + 1) * 512,
                                    ].rearrange("(kt p) s -> p kt s", p=128),
                                )
                        for ft in range(QH + 2):  # 0..3 q heads, 4 k, 5 v
                            acc = ps.tile([128, 512], F32, tag="ad", bufs=4)
                            for kt in range(NKT):
                                if ft < QH:
                                    lhsT = wq_sb[:, kt, ft * D : (ft + 1) * D]
                                elif ft == QH:
                                    lhsT = wk_sb[:, kt, :]
                                else:
                                    lhsT = wv_sb[:, kt, :]
                                nc.tensor.matmul(
                                    acc[:], lhsT, ht[:, kt, :],
                                    start=(kt == 0), stop=(kt == NKT - 1),
                                )
                            xb = p1.tile([128, 512], BF, tag="xb", bufs=4)
                            nc.scalar.activation(xb[:], acc[:], AF.Copy)
                            if pending is not None:
                                finish_tile(*pending)
                            pending = (sc, ft, xb)
                    finish_tile(*pending)

            # ---- phase 2: attention (h outer so A2A can fire per head-pair)
            def attn_head(h, b, qc):
                acc = ps.tile([128, 512], F32, tag="ad", bufs=4)
                den = ps.tile([128, 512], F32, tag="ad", bufs=4)
                qs = h * SG + b * S + qc * 512
                kb = b * S

                # pair list: off-diagonal pairs then the two diagonal pairs.
                # A pair's two score tiles land in one [128,1024] fp32 PSUM
                # tile (2 banks) so ONE exp covers both.
                pairs = [("off", k, k + 1) for k in range(0, 4 * qc, 2)]
                pairs.append(("d0", 4 * qc, 4 * qc + 1))      # widths 512,384
                pairs.append(("d1", 4 * qc + 2, 4 * qc + 3))  # widths 256,128

                def qoff(kt):
                    j = kt - 4 * qc
                    return j * 128 if j > 0 else 0

                def score_pair(p):
                    kind, k0, k1 = p
                    sp = ps.tile(
                        [128, 1024], F32, tag="pair",
                        name=f"s_{h}_{b}_{qc}_{k0}",
                    )
                    o0, o1 = qoff(k0), qoff(k1)
                    # d1 packs both (narrow) tiles into bank 1: second matmul
                    # uses start=False so it doesn't clear the first's data.
                    c1 = 512 - o0 if kind == "d1" else 512
                    nc.tensor.matmul(
                        sp[:, : 512 - o0],
                        kr[:, kb + k0 * 128 : kb + (k0 + 1) * 128],
                        qr[:, qs + o0 : qs + 512],
                    )
                    nc.tensor.matmul(
                        sp[:, c1 : c1 + 512 - o1],
                        kr[:, kb + k1 * 128 : kb + (k1 + 1) * 128],
                        qr[:, qs + o1 : qs + 512],
                        start=(kind != "d1"), stop=True,
                    )
                    return sp

                first_mm = [True]
                first_den = [True]

                def den_mm(src, o, last=False):
                    nc.tensor.matmul(
                        den[:, o:512], ones_mat[:], src,
                        start=first_den[0], stop=last,
                    )
                    first_den[0] = False

                hold = []
                pipe = [score_pair(p) for p in pairs[:2]]
                for i, pr in enumerate(pairs):
                    if i + 2 < len(pairs):
                        pipe.append(score_pair(pairs[i + 2]))
                    sp = pipe.pop(0)
                    kind, k0, k1 = pr
                    pT = sb.tile([128, 1024], BF, tag="pT", bufs=4)
                    if kind == "off":
                        nc.scalar.activation(pT[:], sp[:], AF.Exp)
                        for k, c in ((k0, 0), (k1, 512)):
                            g = b * 16 + k
                            nc.tensor.matmul(
                                acc[:], v_seq[:, g * 128 : (g + 1) * 128],
                                pT[:, c : c + 512],
                                start=first_mm[0], stop=False,
                            )
                            first_mm[0] = False
                        s = sb.tile([128, 512], BF, tag="psum_s", bufs=3)
                        nc.vector.tensor_add(s[:], pT[:, 0:512], pT[:, 512:1024])
                        hold.append(s)
                        if len(hold) == 2:
                            gq = sb.tile([128, 512], BF, tag="psum_g", bufs=2)
                            nc.vector.tensor_add(gq[:], hold[0][:], hold[1][:])
                            den_mm(gq[:], 0)
                            hold = []
                    else:
                        if hold:  # flush leftover off-diag pair-sum
                            den_mm(hold[0][:], 0)
                            hold = []
                        o0, o1 = qoff(k0), qoff(k1)
                        w0, w1 = 512 - o0, 512 - o1
                        if kind == "d0":  # packed at [0:512],[512:896]
                            c1, m0 = 512, 0
                        else:  # d1: packed at [0:256],[256:384] in bank 1
                            c1, m0 = w0, 896
                        w = c1 + w1
                        nc.scalar.activation(pT[:, :w], sp[:, :w], AF.Exp)
                        nc.vector.tensor_mul(
                            pT[:, :w], pT[:, :w], dmask[:, m0 : m0 + w]
                        )
                        for k, c, o, wk_ in ((k0, 0, o0, w0), (k1, c1, o1, w1)):
                            g = b * 16 + k
                            nc.tensor.matmul(
                                acc[:, o:512],
                                v_seq[:, g * 128 : (g + 1) * 128],
                                pT[:, c : c + wk_],
                                start=first_mm[0],
                                stop=(kind == "d1" and k == k1),
                            )
                            first_mm[0] = False
                        # fold k1's P into k0's columns (same q range), then
                        # one denominator matmul for the pair.
                        nc.vector.tensor_add(
                            pT[:, o1 - o0 : w0], pT[:, o1 - o0 : w0],
                            pT[:, c1 : c1 + w1],
                        )
                        den_mm(pT[:, :w0], o0, last=(kind == "d1"))

                # den rows are identical (all-ones stationary) == denominator
                # already broadcast across partitions.
                rb_sb = sb.tile([128, 512], F32, tag="rb_sb")
                nc.vector.reciprocal_approx_fast(rb_sb[:], den[:])
                ao = sb.tile([128, 512], BF, tag="ao", bufs=3)
                nc.vector.tensor_mul(ao[:], acc[:], rb_sb[:])
                sc = b * 4 + qc
                if h < 2:
                    dst = a2a_in[0][sc * 256 + h * 128 : sc * 256 + (h + 1) * 128, :]
                else:
                    dst = a2a_in[h - 1][sc * 128 : (sc + 1) * 128, :]
                nc.sync.dma_start(dst, ao[:])

            with tc.tile_pool(name="wop", bufs=1) as wop, \
                 tc.tile_pool(name="agp", bufs=1) as agp, \
                 tc.tile_pool(name="prt", bufs=1) as prt:
                wo_tiles = []

                def load_wo(t):
                    wt = wop.tile([128, 2048], BF, tag="wo_t", bufs=16)
                    nc.gpsimd.dma_start(wt[:], wo_e[:, t * 2048 : (t + 1) * 2048])
                    wo_tiles.append(wt)

                def a2a(i):
                    nc.gpsimd.collective_compute(
                        "AllToAll",
                        mybir.AluOpType.bypass,
                        replica_groups=[list(range(N_CORES))],
                        ins=[a2a_in[i].opt()],
                        outs=[a2a_out[i].opt()],
                    )

                agt0 = agp.tile([128, 16, 512], BF, tag="ag0")
                agt1 = agp.tile([128, 16, 512], BF, tag="ag1")
                parts = []

                def pass0_chain(hid_t):
                    o_ps = ps.tile([128, 512], F32, tag="ad", bufs=4)
                    for ft in range(16):
                        nc.tensor.matmul(
                            o_ps[:],
                            wo_tiles[hid_t][:, ft * 128 : (ft + 1) * 128],
                            agt0[:, ft, :],
                            start=(ft == 0),
                            stop=(ft == 15),
                        )
                    part = prt.tile([128, 512], BF, tag=f"part{hid_t}")
                    nc.scalar.activation(part[:], o_ps[:], AF.Copy)
                    parts.append(part)

                with nc.named_scope("attn"):
                    # pass-0 o_proj weights prefetch on the idle gpsimd
                    # queue; they land during attention so the PE never
                    # waits at the attention->o_proj boundary.
                    for t in range(16):
                        load_wo(t)
                    for h in (0, 1):
                        for b in range(B):
                            for qc in range(4):
                                attn_head(h, b, qc)
                    a2a(0)
                    # gpsimd (SWDGE) loads: gated on collective 0 only.
                    for ft in range(16):
                        nc.gpsimd.dma_start(
                            agt0[:, ft, :],
                            a2a_out[0][ft * 128 : (ft + 1) * 128, :],
                        )
                    for b in range(B):
                        for qc in range(4):
                            attn_head(2, b, qc)
                    a2a(1)
                    # head 3 attention interleaved with o_proj pass 0: the
                    # pass-0 matmuls fill PE slots while ScalarE catches up
                    # on the remaining exps.
                    p0 = 0
                    for b in range(B):
                        for qc in range(4):
                            attn_head(3, b, qc)
                            pass0_chain(p0)
                            pass0_chain(p0 + 1)
                            p0 += 2
                    a2a(2)

                # ---- phase 4: o_proj pass 1 (features from head-2/3 A2As).
                with nc.named_scope("oproj"):
                    for t in range(16, 32):  # pass-1 weights stream
                        load_wo(t)
                    # agt1 feature blocks alternate (head2, head3) to match
                    # the Wo row order; head-2 blocks load right after
                    # collective 1 completes.
                    for r in range(8):
                        nc.gpsimd.dma_start(
                            agt1[:, 2 * r, :],
                            a2a_out[1][r * 128 : (r + 1) * 128, :],
                        )
                    for r in range(8):
                        nc.gpsimd.dma_start(
                            agt1[:, 2 * r + 1, :],
                            a2a_out[2][r * 128 : (r + 1) * 128, :],
                        )
                    for hid_t in range(NKT):  # 16 tiles of 128 hidden cols
                        wo_t = wo_tiles[16 + hid_t]
                        o_ps = ps.tile([128, 512], F32, tag="ad", bufs=4)
                        for ft in range(16):
                            nc.tensor.matmul(
                                o_ps[:],
                                wo_t[:, ft * 128 : (ft + 1) * 128],
                                agt1[:, ft, :],
                                start=(ft == 0),
                                stop=(ft == 15),
                            )
                        ob = sb.tile([128, 512], F32, tag="ob", bufs=3)
                        nc.vector.tensor_add(ob[:], o_ps[:], parts[hid_t][:])
                        nc.sync.dma_start(
                            outT_e[hid_t * 128 : (hid_t + 1) * 128, :], ob[:]
                        )

    nc.compile()
    return nc


def _prep(hidden_states, sin_table, cos_table, Wq, Wk, Wv, Wo):
    bf = ml_dtypes.bfloat16
    flat = np.asarray(hidden_states, np.float32).reshape(SG, HID)
    hT = np.ascontiguousarray(flat.T).astype(bf)

    cosT = np.asarray(cos_table, np.float32)[:, :64].T  # [64, S]
    sinT = np.asarray(sin_table, np.float32)[:, :64].T
    c_dup = np.tile(np.concatenate([cosT, cosT], 0), (1, B)).astype(bf)
    # sign-folded: rotate-half becomes a plain partition swap
    s_dup = np.tile(np.concatenate([-sinT, sinT], 0), (1, B)).astype(bf)

    kk = np.arange(128)[:, None]
    tril = lambda w: (kk <= np.arange(w)[None, :]).astype(np.float32)
    dmask = np.concatenate(
        [tril(512), tril(384), tril(256), tril(128)], axis=1
    ).astype(bf)

    scale = np.float32(1.0 / np.sqrt(D))
    Wq = np.asarray(Wq, np.float32) * scale
    Wk = np.asarray(Wk, np.float32)
    Wv = np.asarray(Wv, np.float32)
    Wo = np.asarray(Wo, np.float32)

    # Permute Wo rows into the order o_proj consumes the A2A output blocks:
    # a2a1 blocks: (r, h in {0,1}); a2a2 blocks: (r, h in {2,3}). Then lay
    # out as [p, (half, hid_t, ft, c)] so each weight tile is one contiguous
    # [128, 2048] DMA.
    Wo_b = Wo.reshape(H, D, HID)
    order = [4 * r + h for r in range(8) for h in (0, 1)] + [
        4 * r + h for r in range(8) for h in (2, 3)
    ]
    Wo_perm = Wo_b[order].reshape(H * D, HID)
    W5 = Wo_perm.reshape(2, 16, 128, 16, 128)       # [half, ft, p, hid_t, c]
    woL = np.ascontiguousarray(
        W5.transpose(2, 0, 3, 1, 4).reshape(128, 2 * 16 * 16 * 128)
    ).astype(bf)

    in_maps = []
    for c in range(N_CORES):
        in_maps.append(
            {
                "hT": hT,
                "wq": np.ascontiguousarray(Wq[:, c * 512 : (c + 1) * 512]).astype(bf),
                "wk": np.ascontiguousarray(Wk[:, c * D : (c + 1) * D]).astype(bf),
                "wv": np.ascontiguousarray(Wv[:, c * D : (c + 1) * D]).astype(bf),
                "wo": woL,
                "c_dup": c_dup,
                "s_dup": s_dup,
                "dmask": dmask,
            }
        )
    return in_maps


def kernel(**inputs) -> np.ndarray:
    global LAST_EXEC_NS
    if "nc" not in _CACHE:
        _CACHE["nc"] = _build()
    nc = _CACHE["nc"]

    in_maps = _prep(**inputs)
    res = run_bass_kernel_spmd(
        nc,
        in_maps,
        core_ids=list(range(N_CORES)),
        trace=bool(os.environ.get("BASS_TRACE")),
    )
    LAST_EXEC_NS = res.exec_time_ns

    outT = np.concatenate(
        [np.asarray(res.results[c]["outT"], np.float32) for c in range(N_CORES)],
        axis=1,
    )  # [HID, SG]
    return np.ascontiguousarray(outT.T).reshape(B, S, HID)


# revision 20
# speedup vs baseline: 1.0046x; 1.0046x over previous
"""GQA causal attention (B=2, S=2048, HID=2048, H=32, HKV=8, D=128) on 8 TRN2
NeuronCores.

Sharding: tensor-parallel over heads for QKV+attention (core c owns kv head c
and q heads 4c..4c+3), then an AllToAll switches to sequence-parallel for
o_proj (core c computes the full hidden dim for global s-chunk c). The A2A
moves 8x less data than an AllGather and needs no per-core dynamic slicing.
It is split into two collectives (head pairs) so comm overlaps attention
compute of the remaining heads and the first half of o_proj.

Device pipeline (bf16 compute, fp32 PSUM accumulation):
  1. Feature-major projections: Q^T/K^T/V^T = W^T h^T, h^T streamed on the
     sync queue; all small constants stream on the scalar queue so they never
     stall the h^T pipeline.
  2. RoPE as  x*cos_dup + swap_halves(x)*sin_signed  - the rotate-half is a
     pure partition swap done by idle gpsimd SWDGE DMAs (the sign lives in the
     host-prepared sin table); cross-partition DVE ops are illegal.
  3. Transposed flash attention: S^T[k,q] = K^T_chunk.T @ Q^T chunk. Score
     tiles are computed in PAIRS into a 2-bank [128,1024] fp32 PSUM tile so
     one ScalarE exp covers two tiles (the ACT per-op overhead is ~352
     cycles; pairing keeps ACT off the critical path). Causal 0/1 masks for
     the two diagonal pairs are host-packed ([tril512|tril384|tril256|
     tril128]) so each pair needs one DVE mul. The denominator uses an
     all-ones [128,128] stationary (lands pre-broadcast across partitions);
     diagonal-pair P tiles are pre-summed on the DVE so the whole qc needs
     only ~3 denominator matmuls instead of one per tile.
  4. Two AllToAlls (heads 0-1, then 2-3) exchange attn-out^T blocks.
  5. o_proj: out^T[hid, my_s_chunk] accumulated over all 32 feature tiles.
     Wo is host-relaid-out to [128, 32*2048] so each weight tile is one
     contiguous DMA; all pass-0 tiles prefetch on the idle gpsimd queue
     DURING attention, so the PE flows from attention straight into o_proj
     without an idle window (which would re-throttle the PE clock).
Host reassembles the 8 sequence chunks and transposes back.
"""

import os

import numpy as np
import ml_dtypes

from concourse import bacc, mybir
import concourse.tile as tile
from concourse.bass_utils import run_bass_kernel_spmd

N_CORES = 8
B, S, HID = 2, 2048, 2048
H, HKV, D = 32, 8, 128
QH = H // HKV          # q heads per core
SG = B * S             # 4096 global sequence
NSC = SG // 512        # 8 s-chunks of 512
NKT = HID // 128       # 16 hid k-tiles

BF = mybir.dt.bfloat16
F32 = mybir.dt.float32
AF = mybir.ActivationFunctionType

_CACHE = {}
LAST_EXEC_NS = None


def _build():
    nc = bacc.Bacc("TRN2", num_devices=N_CORES)

    hT_e = nc.declare_dram_parameter("hT", [HID, SG], BF, isOutput=False)
    wq_e = nc.declare_dram_parameter("wq", [HID, QH * D], BF, isOutput=False)
    wk_e = nc.declare_dram_parameter("wk", [HID, D], BF, isOutput=False)
    wv_e = nc.declare_dram_parameter("wv", [HID, D], BF, isOutput=False)
    # Wo host-relaid-out: [p, (half, hid_t, ft, c)] so each [128, 2048] slice
    # is one o_proj weight tile, contiguous per partition.
    wo_e = nc.declare_dram_parameter("wo", [128, 32 * 16 * 128], BF, isOutput=False)
    cd_e = nc.declare_dram_parameter("c_dup", [D, SG], BF, isOutput=False)
    sd_e = nc.declare_dram_parameter("s_dup", [D, SG], BF, isOutput=False)
    # packed diagonal-pair masks: [tril512 | tril384 | tril256 | tril128]
    dm_e = nc.declare_dram_parameter("dmask", [128, 1280], BF, isOutput=False)
    outT_e = nc.declare_dram_parameter("outT", [HID, 512], F32, isOutput=True)

    with tile.TileContext(nc) as tc:
        with (
            tc.tile_pool(name="cst", bufs=1) as cst,
            tc.tile_pool(name="sb", bufs=2) as sb,
            tc.tile_pool(name="ps", bufs=2, space="PSUM") as ps,
            tc.tile_pool(name="dram", bufs=1, space="DRAM") as dram,
        ):
            dmask = cst.tile([128, 1280], BF, tag="dmask")
            ones_mat = cst.tile([128, 128], BF, tag="ones_mat")
            nc.gpsimd.memset(ones_mat[:], 1.0)

            qr = cst.tile([128, QH * SG], BF, tag="qr")
            kr = cst.tile([128, SG], BF, tag="kr")
            v_seq = cst.tile([128, SG], BF, tag="v_seq")

            # A2A bounce buffers. Collective 0 carries heads {0,1} (shard j =
            # 256 rows), collectives 1/2 carry head 2 / head 3 (shard j = 128
            # rows) so the last collective is small and lands well before the
            # o_proj second pass needs it.
            a2a_rows = (256, 128, 128)
            a2a_in = [
                dram.tile([8 * r, 512], BF, name=f"a2ain{i}", tag=f"a2ain{i}")
                for i, r in enumerate(a2a_rows)
            ]
            a2a_out = [
                dram.tile([8 * r, 512], BF, name=f"a2aout{i}", tag=f"a2aout{i}")
                for i, r in enumerate(a2a_rows)
            ]

            # ---- phase 1: projections + rope + V transpose ----
            with tc.tile_pool(name="p1", bufs=1) as p1, \
                 tc.tile_pool(name="htp", bufs=3) as htp:
                # piece-wise loads on the sync queue: per-kt pieces give
                # per-region deps, so the first matmul starts after ~256KB.
                # Everything else rides the scalar queue so the sync queue is
                # a pure wq/hT stream.
                wq_sb = p1.tile([128, NKT, QH * D], BF, tag="wq_sb")
                ht0 = htp.tile([128, NKT, 512], BF, tag="ht")
                c_d = p1.tile([D, SG], BF, tag="c_d")
                s_d = p1.tile([D, SG], BF, tag="s_d")
                # coarse 4-kt pieces: the Sync engine pays ~0.7us issue time
                # per DMA instruction, so fewer+bigger beats 32 small pieces.
                for kq in range(4):
                    nc.sync.dma_start(
                        wq_sb[:, 4 * kq : 4 * kq + 4, :],
                        wq_e[4 * kq * 128 : (4 * kq + 4) * 128, :].rearrange(
                            "(kt p) f -> p kt f", p=128
                        ),
                    )
                    nc.sync.dma_start(
                        ht0[:, 4 * kq : 4 * kq + 4, :],
                        hT_e[
                            4 * kq * 128 : (4 * kq + 4) * 128, 0:512
                        ].rearrange("(kt p) s -> p kt s", p=128),
                    )
                    if kq == 0:
                        # rope constants for the first chunk, off-queue
                        nc.scalar.dma_start(c_d[:, 0:512], cd_e[:, 0:512])
                        nc.scalar.dma_start(s_d[:, 0:512], sd_e[:, 0:512])
                wk_sb = p1.tile([128, NKT, D], BF, tag="wk_sb")
                nc.scalar.dma_start(
                    wk_sb[:], wk_e[:].rearrange("(kt p) f -> p kt f", p=128)
                )
                wv_sb = p1.tile([128, NKT, D], BF, tag="wv_sb")
                nc.scalar.dma_start(
                    wv_sb[:], wv_e[:].rearrange("(kt p) f -> p kt f", p=128)
                )
                nc.scalar.dma_start(dmask[:], dm_e[:])
                for sc in range(1, NSC):
                    nc.scalar.dma_start(
                        c_d[:, sc * 512 : (sc + 1) * 512],
                        cd_e[:, sc * 512 : (sc + 1) * 512],
                    )
                    nc.scalar.dma_start(
                        s_d[:, sc * 512 : (sc + 1) * 512],
                        sd_e[:, sc * 512 : (sc + 1) * 512],
                    )

                # rope/V-transpose for tile i are emitted AFTER projection
                # chain i+1 so their PE ops never wait on the ACT evacuation.
                def finish_tile(sc, ft, xb):
                    if ft < QH + 1:  # rope for q heads and k
                        # rotate-half = partition swap via idle gpsimd SWDGE
                        # (sin table sign-folded on host)
                        sh = p1.tile([128, 512], BF, tag="sh", bufs=3)
                        nc.gpsimd.dma_start(sh[0:64, :], xb[64:128, :])
                        nc.gpsimd.dma_start(sh[64:128, :], xb[0:64, :])
                        if ft < QH:
                            dest = qr[
                                :, ft * SG + sc * 512 : ft * SG + sc * 512 + 512
                            ]
                        else:
                            dest = kr[:, sc * 512 : sc * 512 + 512]
                        cs = c_d[:, sc * 512 : (sc + 1) * 512]
                        ss = s_d[:, sc * 512 : (sc + 1) * 512]
                        nc.vector.tensor_mul(dest, xb[:], cs)
                        rtmp = p1.tile([128, 512], BF, tag="rtmp", bufs=2)
                        nc.vector.tensor_mul(rtmp[:], sh[:], ss)
                        nc.vector.tensor_add(dest, dest, rtmp[:])
                    else:  # v: transpose to seq-major via the DMA xbar —
                        # keeps the 32 transposes off the TensorE FIFO.
                        for j in range(4):
                            g = sc * 4 + j
                            nc.scalar.dma_start(
                                v_seq[:, g * 128 : (g + 1) * 128],
                                xb[:, j * 128 : (j + 1) * 128],
                                transpose=True,
                            )

                def load_ht(sc):
                    ht = htp.tile([128, NKT, 512], BF, tag="ht")
                    for kq in range(4):  # 4 coarser pieces
                        nc.sync.dma_start(
                            ht[:, 4 * kq : 4 * kq + 4, :],
                            hT_e[
                                4 * kq * 128 : (4 * kq + 4) * 128,
                                sc * 512 : (sc + 1) * 512,
                            ].rearrange("(kt p) s -> p kt s", p=128),
                        )
                    return ht

                # keep the double-buffer primed two chunks ahead
                ht_pre = {0: ht0, 1: load_ht(1), 2: load_ht(2)}

                with nc.named_scope("proj"):
                    pending = None
                    for sc in range(NSC):
                        if sc in ht_pre:
                            ht = ht_pre[sc]
                        else:
                            ht = load_ht(sc)
                        for ft in range(QH + 2):  # 0..3 q heads, 4 k, 5 v
                            acc = ps.tile([128, 512], F32, tag="ad", bufs=4)
                            for kt in range(NKT):
                                if ft < QH:
                                    lhsT = wq_sb[:, kt, ft * D : (ft + 1) * D]
                                elif ft == QH:
                                    lhsT = wk_sb[:, kt, :]
                                else:
                                    lhsT = wv_sb[:, kt, :]
                                nc.tensor.matmul(
                                    acc[:], lhsT, ht[:, kt, :],
                                    start=(kt == 0), stop=(kt == NKT - 1),
                                )
                            xb = p1.tile([128, 512], BF, tag="xb", bufs=4)
                            nc.scalar.activation(xb[:], acc[:], AF.Copy)
                            if pending is not None:
                                finish_tile(*pending)
                            pending = (sc, ft, xb)
                    finish_tile(*pending)

            # ---- phase 2: attention (h outer so A2A can fire per head-pair)
            def attn_head(h, b, qc):
                acc = ps.tile([128, 512], F32, tag="ad", bufs=4)
                den = ps.tile([128, 512], F32, tag="ad", bufs=4)
                qs = h * SG + b * S + qc * 512
                kb = b * S

                # pair list: off-diagonal pairs then the two diagonal pairs.
                # A pair's two score tiles land in one [128,1024] fp32 PSUM
                # tile (2 banks) so ONE exp covers both.
                pairs = [("off", k, k + 1) for k in range(0, 4 * qc, 2)]
                pairs.append(("d0", 4 * qc, 4 * qc + 1))      # widths 512,384
                pairs.append(("d1", 4 * qc + 2, 4 * qc + 3))  # widths 256,128

                def qoff(kt):
                    j = kt - 4 * qc
                    return j * 128 if j > 0 else 0

                def score_pair(p):
                    kind, k0, k1 = p
                    sp = ps.tile(
                        [128, 1024], F32, tag="pair",
                        name=f"s_{h}_{b}_{qc}_{k0}",
                    )
                    o0, o1 = qoff(k0), qoff(k1)
                    # d1 packs both (narrow) tiles into bank 1: second matmul
                    # uses start=False so it doesn't clear the first's data.
                    c1 = 512 - o0 if kind == "d1" else 512
                    nc.tensor.matmul(
                        sp[:, : 512 - o0],
                        kr[:, kb + k0 * 128 : kb + (k0 + 1) * 128],
                        qr[:, qs + o0 : qs + 512],
                    )
                    nc.tensor.matmul(
                        sp[:, c1 : c1 + 512 - o1],
                        kr[:, kb + k1 * 128 : kb + (k1 + 1) * 128],
                        qr[:, qs + o1 : qs + 512],
                        start=(kind != "d1"), stop=True,
                    )
                    return sp

                first_mm = [True]
                first_den = [True]

                def den_mm(src, o, last=False):
                    nc.tensor.matmul(
                        den[:, o:512], ones_mat[:], src,
                        start=first_den[0], stop=last,
                    )
                    first_den[0] = False

                hold = []
                pipe = [score_pair(p) for p in pairs[:2]]
                for i, pr in enumerate(pairs):
                    if i + 2 < len(pairs):
                        pipe.append(score_pair(pairs[i + 2]))
                    sp = pipe.pop(0)
                    kind, k0, k1 = pr
                    pT = sb.tile([128, 1024], BF, tag="pT", bufs=4)
                    if kind == "off":
                        nc.scalar.activation(pT[:], sp[:], AF.Exp)
                        for k, c in ((k0, 0), (k1, 512)):
                            g = b * 16 + k
                            nc.tensor.matmul(
                                acc[:], v_seq[:, g * 128 : (g + 1) * 128],
                                pT[:, c : c + 512],
                                start=first_mm[0], stop=False,
                            )
                            first_mm[0] = False
                        s = sb.tile([128, 512], BF, tag="psum_s", bufs=3)
                        nc.vector.tensor_add(s[:], pT[:, 0:512], pT[:, 512:1024])
                        hold.append(s)
                        if len(hold) == 2:
                            gq = sb.tile([128, 512], BF, tag="psum_g", bufs=2)
                            nc.vector.tensor_add(gq[:], hold[0][:], hold[1][:])
                            den_mm(gq[:], 0)
                            hold = []
                    else:
                        if hold:  # flush leftover off-diag pair-sum
                            den_mm(hold[0][:], 0)
                            hold = []
                        o0, o1 = qoff(k0), qoff(k1)
                        w0, w1 = 512 - o0, 512 - o1
                        if kind == "d0":  # packed at [0:512],[512:896]
                            c1, m0 = 512, 0
                        else:  # d1: packed at [0:256],[256:384] in bank 1
                            c1, m0 = w0, 896
                        w = c1 + w1
                        nc.scalar.activation(pT[:, :w], sp[:, :w], AF.Exp)
                        nc.vector.tensor_mul(
                            pT[:, :w], pT[:, :w], dmask[:, m0 : m0 + w]
                        )
                        for k, c, o, wk_ in ((k0, 0, o0, w0), (k1, c1, o1, w1)):
                            g = b * 16 + k
                            nc.tensor.matmul(
                                acc[:, o:512],
                                v_seq[:, g * 128 : (g + 1) * 128],
                                pT[:, c : c + wk_],
                                start=first_mm[0],
                                stop=(kind == "d1" and k == k1),
                            )
                            first_mm[0] = False
                        # fold k1's P into k0's columns (same q range), then
                        # one denominator matmul for the pair.
                        nc.vector.tensor_add(
                            pT[:, o1 - o0 : w0], pT[:, o1 - o0 : w0],
                            pT[:, c1 : c1 + w1],
                        )
                        den_mm(pT[:, :w0], o0, last=(kind == "d1"))

                # den rows are identical (all-ones stationary) == denominator
                # already broadcast across partitions.
                rb_sb = sb.tile([128, 512], F32, tag="rb_sb")
                nc.vector.reciprocal_approx_fast(rb_sb[:], den[:])
                ao = sb.tile([128, 512], BF, tag="ao", bufs=3)
                nc.vector.tensor_mul(ao[:], acc[:], rb_sb[:])
                sc = b * 4 + qc
                if h < 2:
                    dst = a2a_in[0][sc * 256 + h * 128 : sc * 256 + (h + 1) * 128, :]
                else:
                    dst = a2a_in[h - 1][sc * 128 : (sc + 1) * 128, :]
                nc.sync.dma_start(dst, ao[:])

            with tc.tile_pool(name="wop", bufs=1) as wop, \
                 tc.tile_pool(name="agp", bufs=1) as agp, \
                 tc.tile_pool(name="prt", bufs=1) as prt:
                wo_tiles = []

                def load_wo(t):
                    wt = wop.tile([128, 2048], BF, tag="wo_t", bufs=16)
                    nc.gpsimd.dma_start(wt[:], wo_e[:, t * 2048 : (t + 1) * 2048])
                    wo_tiles.append(wt)

                def a2a(i):
                    nc.gpsimd.collective_compute(
                        "AllToAll",
                        mybir.AluOpType.bypass,
                        replica_groups=[list(range(N_CORES))],
                        ins=[a2a_in[i].opt()],
                        outs=[a2a_out[i].opt()],
                    )

                agt0 = agp.tile([128, 16, 512], BF, tag="ag0")
                agt1 = agp.tile([128, 16, 512], BF, tag="ag1")
                parts = []

                def pass0_chain(hid_t):
                    o_ps = ps.tile([128, 512], F32, tag="ad", bufs=4)
                    for ft in range(16):
                        nc.tensor.matmul(
                            o_ps[:],
                            wo_tiles[hid_t][:, ft * 128 : (ft + 1) * 128],
                            agt0[:, ft, :],
                            start=(ft == 0),
                            stop=(ft == 15),
                        )
                    part = prt.tile([128, 512], BF, tag=f"part{hid_t}")
                    nc.scalar.activation(part[:], o_ps[:], AF.Copy)
                    parts.append(part)

                with nc.named_scope("attn"):
                    # pass-0 o_proj weights prefetch on the idle gpsimd
                    # queue; they land during attention so the PE never
                    # waits at the attention->o_proj boundary.
                    for t in range(16):
                        load_wo(t)
                    for h in (0, 1):
                        for b in range(B):
                            for qc in range(4):
                                attn_head(h, b, qc)
                    a2a(0)
                    # single big gather: gated on collective 0 only
                    nc.gpsimd.dma_start(
                        agt0[:],
                        a2a_out[0][:].rearrange("(ft p) s -> p ft s", p=128),
                    )
                    for b in range(B):
                        for qc in range(4):
                            attn_head(2, b, qc)
                    a2a(1)
                    # head-2 feature blocks land in agt1's first half
                    nc.gpsimd.dma_start(
                        agt1[:, 0:8, :],
                        a2a_out[1][:].rearrange("(ft p) s -> p ft s", p=128),
                    )
                    # head-3 b=0 attention interleaved with the first half of
                    # o_proj pass 0: the pass-0 matmuls fill PE slots while
                    # ScalarE catches up on the exps. (Not with b=1: pass-0
                    # work before head-3's last store would delay the last
                    # collective, which gates pass 1.)
                    p0 = 0
                    for qc in range(4):
                        attn_head(3, 0, qc)
                        pass0_chain(p0)
                        pass0_chain(p0 + 1)
                        p0 += 2
                    for qc in range(4):
                        attn_head(3, 1, qc)
                    a2a(2)

                # ---- phase 4: o_proj pass 1 (features from head-2/3 A2As).
                with nc.named_scope("oproj"):
                    for t in range(16, 32):  # pass-1 weights stream
                        load_wo(t)
                    nc.gpsimd.dma_start(
                        agt1[:, 8:16, :],
                        a2a_out[2][:].rearrange("(ft p) s -> p ft s", p=128),
                    )
                    for hid_t in range(8, 16):  # remaining pass-0 chains
                        pass0_chain(hid_t)
                    for hid_t in range(NKT):  # 16 tiles of 128 hidden cols
                        wo_t = wo_tiles[16 + hid_t]
                        o_ps = ps.tile([128, 512], F32, tag="ad", bufs=4)
                        for ft in range(16):
                            nc.tensor.matmul(
                                o_ps[:],
                                wo_t[:, ft * 128 : (ft + 1) * 128],
                                agt1[:, ft, :],
                                start=(ft == 0),
                                stop=(ft == 15),
                            )
                        ob = sb.tile([128, 512], F32, tag="ob", bufs=3)
                        nc.vector.tensor_add(ob[:], o_ps[:], parts[hid_t][:])
                        nc.sync.dma_start(
                            outT_e[hid_t * 128 : (hid_t + 1) * 128, :], ob[:]
                        )

    nc.compile()
    return nc


def _prep(hidden_states, sin_table, cos_table, Wq, Wk, Wv, Wo):
    bf = ml_dtypes.bfloat16
    flat = np.asarray(hidden_states, np.float32).reshape(SG, HID)
    hT = np.ascontiguousarray(flat.T).astype(bf)

    cosT = np.asarray(cos_table, np.float32)[:, :64].T  # [64, S]
    sinT = np.asarray(sin_table, np.float32)[:, :64].T
    c_dup = np.tile(np.concatenate([cosT, cosT], 0), (1, B)).astype(bf)
    # sign-folded: rotate-half becomes a plain partition swap
    s_dup = np.tile(np.concatenate([-sinT, sinT], 0), (1, B)).astype(bf)

    kk = np.arange(128)[:, None]
    tril = lambda w: (kk <= np.arange(w)[None, :]).astype(np.float32)
    dmask = np.concatenate(
        [tril(512), tril(384), tril(256), tril(128)], axis=1
    ).astype(bf)

    scale = np.float32(1.0 / np.sqrt(D))
    Wq = np.asarray(Wq, np.float32) * scale
    Wk = np.asarray(Wk, np.float32)
    Wv = np.asarray(Wv, np.float32)
    Wo = np.asarray(Wo, np.float32)

    # Permute Wo rows into the order o_proj consumes the A2A output blocks:
    # a2a1 blocks: (r, h in {0,1}); a2a2 blocks: (r, h in {2,3}). Then lay
    # out as [p, (half, hid_t, ft, c)] so each weight tile is one contiguous
    # [128, 2048] DMA.
    Wo_b = Wo.reshape(H, D, HID)
    order = (
        [4 * r + h for r in range(8) for h in (0, 1)]
        + [4 * r + 2 for r in range(8)]
        + [4 * r + 3 for r in range(8)]
    )
    Wo_perm = Wo_b[order].reshape(H * D, HID)
    W5 = Wo_perm.reshape(2, 16, 128, 16, 128)       # [half, ft, p, hid_t, c]
    woL = np.ascontiguousarray(
        W5.transpose(2, 0, 3, 1, 4).reshape(128, 2 * 16 * 16 * 128)
    ).astype(bf)

    in_maps = []
    for c in range(N_CORES):
        in_maps.append(
            {
                "hT": hT,
                "wq": np.ascontiguousarray(Wq[:, c * 512 : (c + 1) * 512]).astype(bf),
                "wk": np.ascontiguousarray(Wk[:, c * D : (c + 1) * D]).astype(bf),
                "wv": np.ascontiguousarray(Wv[:, c * D : (c + 1) * D]).astype(bf),
                "wo": woL,
                "c_dup": c_dup,
                "s_dup": s_dup,
                "dmask": dmask,
            }
        )
    return in_maps


def kernel(**inputs) -> np.ndarray:
    global LAST_EXEC_NS
    if "nc" not in _CACHE:
        _CACHE["nc"] = _build()
    nc = _CACHE["nc"]

    in_maps = _prep(**inputs)
    res = run_bass_kernel_spmd(
        nc,
        in_maps,
        core_ids=list(range(N_CORES)),
        trace=bool(os.environ.get("BASS_TRACE")),
    )
    LAST_EXEC_NS = res.exec_time_ns

    outT = np.concatenate(
        [np.asarray(res.results[c]["outT"], np.float32) for c in range(N_CORES)],
        axis=1,
    )  # [HID, SG]
    return np.ascontiguousarray(outT.T).reshape(B, S, HID)


# revision 22
# speedup vs baseline: 1.0983x; 1.0933x over previous
"""GQA causal attention (B=2, S=2048, HID=2048, H=32, HKV=8, D=128) on 8 TRN2
NeuronCores.

Sharding: tensor-parallel over heads for QKV+attention (core c owns kv head c
and q heads 4c..4c+3), then an AllToAll switches to sequence-parallel for
o_proj (core c computes the full hidden dim for global s-chunk c). The A2A
moves 8x less data than an AllGather and needs no per-core dynamic slicing.
It is split into two collectives (head pairs) so comm overlaps attention
compute of the remaining heads and the first half of o_proj.

Device pipeline (bf16 compute, fp32 PSUM accumulation):
  1. Feature-major projections: Q^T/K^T/V^T = W^T h^T, h^T streamed on the
     sync queue; all small constants stream on the scalar queue so they never
     stall the h^T pipeline.
  2. RoPE as  x*cos_dup + swap_halves(x)*sin_signed  - the rotate-half is a
     pure partition swap done by idle gpsimd SWDGE DMAs (the sign lives in the
     host-prepared sin table); cross-partition DVE ops are illegal.
  3. Transposed flash attention: S^T[k,q] = K^T_chunk.T @ Q^T chunk. Score
     tiles are computed in PAIRS into a 2-bank [128,1024] fp32 PSUM tile so
     one ScalarE exp covers two tiles (the ACT per-op overhead is ~352
     cycles; pairing keeps ACT off the critical path). Causal 0/1 masks for
     the two diagonal pairs are host-packed ([tril512|tril384|tril256|
     tril128]) so each pair needs one DVE mul. The denominator uses an
     all-ones [128,128] stationary (lands pre-broadcast across partitions);
     diagonal-pair P tiles are pre-summed on the DVE so the whole qc needs
     only ~3 denominator matmuls instead of one per tile.
  4. Two AllToAlls (heads 0-1, then 2-3) exchange attn-out^T blocks.
  5. o_proj: out^T[hid, my_s_chunk] accumulated over all 32 feature tiles.
     Wo is host-relaid-out to [128, 32*2048] so each weight tile is one
     contiguous DMA; all pass-0 tiles prefetch on the idle gpsimd queue
     DURING attention, so the PE flows from attention straight into o_proj
     without an idle window (which would re-throttle the PE clock).
Host reassembles the 8 sequence chunks and transposes back.
"""

import os

import numpy as np
import ml_dtypes

from concourse import bacc, mybir
import concourse.tile as tile
from concourse.bass_utils import run_bass_kernel_spmd

N_CORES = 8
B, S, HID = 2, 2048, 2048
H, HKV, D = 32, 8, 128
QH = H // HKV          # q heads per core
SG = B * S             # 4096 global sequence
NSC = SG // 512        # 8 s-chunks of 512
NKT = HID // 128       # 16 hid k-tiles

BF = mybir.dt.bfloat16
F32 = mybir.dt.float32
AF = mybir.ActivationFunctionType

_CACHE = {}
LAST_EXEC_NS = None


def _build():
    nc = bacc.Bacc("TRN2", num_devices=N_CORES)

    hT_e = nc.declare_dram_parameter("hT", [HID, SG], BF, isOutput=False)
    wq_e = nc.declare_dram_parameter("wq", [HID, QH * D], BF, isOutput=False)
    wk_e = nc.declare_dram_parameter("wk", [HID, D], BF, isOutput=False)
    wv_e = nc.declare_dram_parameter("wv", [HID, D], BF, isOutput=False)
    # Wo host-relaid-out: [p, (half, hid_t, ft, c)] so each [128, 2048] slice
    # is one o_proj weight tile, contiguous per partition.
    wo_e = nc.declare_dram_parameter("wo", [128, 32 * 16 * 128], BF, isOutput=False)
    cd_e = nc.declare_dram_parameter("c_dup", [D, SG], BF, isOutput=False)
    sd_e = nc.declare_dram_parameter("s_dup", [D, SG], BF, isOutput=False)
    # packed diagonal-pair masks: [tril512 | tril384 | tril256 | tril128]
    dm_e = nc.declare_dram_parameter("dmask", [128, 1280], BF, isOutput=False)
    outT_e = nc.declare_dram_parameter("outT", [HID, 512], F32, isOutput=True)

    with tile.TileContext(nc) as tc:
        with (
            tc.tile_pool(name="cst", bufs=1) as cst,
            tc.tile_pool(name="sb", bufs=2) as sb,
            tc.tile_pool(name="ps", bufs=2, space="PSUM") as ps,
            tc.tile_pool(name="dram", bufs=1, space="DRAM") as dram,
        ):
            dmask = cst.tile([128, 1280], BF, tag="dmask")
            ones_mat = cst.tile([128, 128], BF, tag="ones_mat")
            nc.gpsimd.memset(ones_mat[:], 1.0)

            qr = cst.tile([128, QH * SG], BF, tag="qr")
            kr = cst.tile([128, SG], BF, tag="kr")
            v_seq = cst.tile([128, SG], BF, tag="v_seq")

            # A2A bounce buffers. Collective 0 carries heads {0,1} (shard j =
            # 256 rows), collectives 1/2 carry head 2 / head 3 (shard j = 128
            # rows) so the last collective is small and lands well before the
            # o_proj second pass needs it.
            a2a_rows = (256, 128, 128)
            a2a_in = [
                dram.tile([8 * r, 512], BF, name=f"a2ain{i}", tag=f"a2ain{i}")
                for i, r in enumerate(a2a_rows)
            ]
            a2a_out = [
                dram.tile([8 * r, 512], BF, name=f"a2aout{i}", tag=f"a2aout{i}")
                for i, r in enumerate(a2a_rows)
            ]

            # ---- phase 1: projections + rope + V transpose ----
            with tc.tile_pool(name="p1", bufs=1) as p1, \
                 tc.tile_pool(name="htp", bufs=3) as htp:
                # piece-wise loads on the sync queue: per-kt pieces give
                # per-region deps, so the first matmul starts after ~256KB.
                # Everything else rides the scalar queue so the sync queue is
                # a pure wq/hT stream.
                wq_sb = p1.tile([128, NKT, QH * D], BF, tag="wq_sb")
                ht0 = htp.tile([128, NKT, 512], BF, tag="ht")
                c_d = p1.tile([D, SG], BF, tag="c_d")
                s_d = p1.tile([D, SG], BF, tag="s_d")
                # coarse 4-kt pieces: the Sync engine pays ~0.7us issue time
                # per DMA instruction, so fewer+bigger beats 32 small pieces.
                for kq in range(4):
                    nc.sync.dma_start(
                        wq_sb[:, 4 * kq : 4 * kq + 4, :],
                        wq_e[4 * kq * 128 : (4 * kq + 4) * 128, :].rearrange(
                            "(kt p) f -> p kt f", p=128
                        ),
                    )
                    nc.sync.dma_start(
                        ht0[:, 4 * kq : 4 * kq + 4, :],
                        hT_e[
                            4 * kq * 128 : (4 * kq + 4) * 128, 0:512
                        ].rearrange("(kt p) s -> p kt s", p=128),
                    )
                    if kq == 0:
                        # rope constants for the first chunk, off-queue
                        nc.scalar.dma_start(c_d[:, 0:512], cd_e[:, 0:512])
                        nc.scalar.dma_start(s_d[:, 0:512], sd_e[:, 0:512])
                wk_sb = p1.tile([128, NKT, D], BF, tag="wk_sb")
                nc.scalar.dma_start(
                    wk_sb[:], wk_e[:].rearrange("(kt p) f -> p kt f", p=128)
                )
                wv_sb = p1.tile([128, NKT, D], BF, tag="wv_sb")
                nc.scalar.dma_start(
                    wv_sb[:], wv_e[:].rearrange("(kt p) f -> p kt f", p=128)
                )
                nc.scalar.dma_start(dmask[:], dm_e[:])
                for sc in range(1, NSC):
                    nc.scalar.dma_start(
                        c_d[:, sc * 512 : (sc + 1) * 512],
                        cd_e[:, sc * 512 : (sc + 1) * 512],
                    )
                    nc.scalar.dma_start(
                        s_d[:, sc * 512 : (sc + 1) * 512],
                        sd_e[:, sc * 512 : (sc + 1) * 512],
                    )

                # rope/V-transpose for tile i are emitted AFTER projection
                # chain i+1 so their PE ops never wait on the ACT evacuation.
                def finish_tile(sc, ft, xb):
                    if ft < QH + 1:  # rope for q heads and k
                        # rotate-half = partition swap via idle gpsimd SWDGE
                        # (sin table sign-folded on host)
                        sh = p1.tile([128, 512], BF, tag="sh", bufs=3)
                        nc.gpsimd.dma_start(sh[0:64, :], xb[64:128, :])
                        nc.gpsimd.dma_start(sh[64:128, :], xb[0:64, :])
                        if ft < QH:
                            dest = qr[
                                :, ft * SG + sc * 512 : ft * SG + sc * 512 + 512
                            ]
                        else:
                            dest = kr[:, sc * 512 : sc * 512 + 512]
                        cs = c_d[:, sc * 512 : (sc + 1) * 512]
                        ss = s_d[:, sc * 512 : (sc + 1) * 512]
                        nc.vector.tensor_mul(dest, xb[:], cs)
                        rtmp = p1.tile([128, 512], BF, tag="rtmp", bufs=2)
                        nc.vector.tensor_mul(rtmp[:], sh[:], ss)
                        nc.vector.tensor_add(dest, dest, rtmp[:])
                    else:  # v: transpose to seq-major via the DMA xbar —
                        # keeps the 32 transposes off the TensorE FIFO.
                        for j in range(4):
                            g = sc * 4 + j
                            nc.scalar.dma_start(
                                v_seq[:, g * 128 : (g + 1) * 128],
                                xb[:, j * 128 : (j + 1) * 128],
                                transpose=True,
                            )

                def load_ht(sc):
                    ht = htp.tile([128, NKT, 512], BF, tag="ht")
                    for kq in range(4):  # 4 coarser pieces
                        nc.sync.dma_start(
                            ht[:, 4 * kq : 4 * kq + 4, :],
                            hT_e[
                                4 * kq * 128 : (4 * kq + 4) * 128,
                                sc * 512 : (sc + 1) * 512,
                            ].rearrange("(kt p) s -> p kt s", p=128),
                        )
                    return ht

                # keep the double-buffer primed two chunks ahead
                ht_pre = {0: ht0, 1: load_ht(1), 2: load_ht(2)}

                with nc.named_scope("proj"):
                    pending = None
                    for sc in range(NSC):
                        if sc in ht_pre:
                            ht = ht_pre[sc]
                        else:
                            ht = load_ht(sc)
                        for ft in range(QH + 2):  # 0..3 q heads, 4 k, 5 v
                            acc = ps.tile([128, 512], F32, tag="ad", bufs=4)
                            for kt in range(NKT):
                                if ft < QH:
                                    lhsT = wq_sb[:, kt, ft * D : (ft + 1) * D]
                                elif ft == QH:
                                    lhsT = wk_sb[:, kt, :]
                                else:
                                    lhsT = wv_sb[:, kt, :]
                                nc.tensor.matmul(
                                    acc[:], lhsT, ht[:, kt, :],
                                    start=(kt == 0), stop=(kt == NKT - 1),
                                )
                            xb = p1.tile([128, 512], BF, tag="xb", bufs=4)
                            nc.scalar.activation(xb[:], acc[:], AF.Copy)
                            if pending is not None:
                                finish_tile(*pending)
                            pending = (sc, ft, xb)
                    finish_tile(*pending)

            # ---- phase 2: attention (h outer so A2A can fire per head-pair)
            def attn_head(h, b, qc):
                acc = ps.tile([128, 512], F32, tag="ad", bufs=4)
                den = ps.tile([128, 512], F32, tag="ad", bufs=4)
                qs = h * SG + b * S + qc * 512
                kb = b * S

                # pair list: off-diagonal pairs then the two diagonal pairs.
                # A pair's two score tiles land in one [128,1024] fp32 PSUM
                # tile (2 banks) so ONE exp covers both.
                pairs = [("off", k, k + 1) for k in range(0, 4 * qc, 2)]
                pairs.append(("d0", 4 * qc, 4 * qc + 1))      # widths 512,384
                pairs.append(("d1", 4 * qc + 2, 4 * qc + 3))  # widths 256,128

                def qoff(kt):
                    j = kt - 4 * qc
                    return j * 128 if j > 0 else 0

                def score_pair(p):
                    kind, k0, k1 = p
                    sp = ps.tile(
                        [128, 1024], F32, tag="pair",
                        name=f"s_{h}_{b}_{qc}_{k0}",
                    )
                    o0, o1 = qoff(k0), qoff(k1)
                    # d1 packs both (narrow) tiles into bank 1: second matmul
                    # uses start=False so it doesn't clear the first's data.
                    c1 = 512 - o0 if kind == "d1" else 512
                    nc.tensor.matmul(
                        sp[:, : 512 - o0],
                        kr[:, kb + k0 * 128 : kb + (k0 + 1) * 128],
                        qr[:, qs + o0 : qs + 512],
                    )
                    nc.tensor.matmul(
                        sp[:, c1 : c1 + 512 - o1],
                        kr[:, kb + k1 * 128 : kb + (k1 + 1) * 128],
                        qr[:, qs + o1 : qs + 512],
                        start=(kind != "d1"), stop=True,
                    )
                    return sp

                first_mm = [True]
                first_den = [True]

                def den_mm(src, o, last=False):
                    nc.tensor.matmul(
                        den[:, o:512], ones_mat[:], src,
                        start=first_den[0], stop=last,
                    )
                    first_den[0] = False

                hold = []
                pipe = [score_pair(p) for p in pairs[:2]]
                for i, pr in enumerate(pairs):
                    if i + 2 < len(pairs):
                        pipe.append(score_pair(pairs[i + 2]))
                    sp = pipe.pop(0)
                    kind, k0, k1 = pr
                    pT = sb.tile([128, 1024], BF, tag="pT", bufs=4)
                    if kind == "off":
                        nc.scalar.activation(pT[:], sp[:], AF.Exp)
                        for k, c in ((k0, 0), (k1, 512)):
                            g = b * 16 + k
                            nc.tensor.matmul(
                                acc[:], v_seq[:, g * 128 : (g + 1) * 128],
                                pT[:, c : c + 512],
                                start=first_mm[0], stop=False,
                            )
                            first_mm[0] = False
                        s = sb.tile([128, 512], BF, tag="psum_s", bufs=3)
                        nc.vector.tensor_add(s[:], pT[:, 0:512], pT[:, 512:1024])
                        hold.append(s)
                        if len(hold) == 2:
                            gq = sb.tile([128, 512], BF, tag="psum_g", bufs=2)
                            nc.vector.tensor_add(gq[:], hold[0][:], hold[1][:])
                            den_mm(gq[:], 0)
                            hold = []
                    else:
                        if hold:  # flush leftover off-diag pair-sum
                            den_mm(hold[0][:], 0)
                            hold = []
                        o0, o1 = qoff(k0), qoff(k1)
                        w0, w1 = 512 - o0, 512 - o1
                        if kind == "d0":  # packed at [0:512],[512:896]
                            c1, m0 = 512, 0
                        else:  # d1: packed at [0:256],[256:384] in bank 1
                            c1, m0 = w0, 896
                        w = c1 + w1
                        nc.scalar.activation(pT[:, :w], sp[:, :w], AF.Exp)
                        nc.vector.tensor_mul(
                            pT[:, :w], pT[:, :w], dmask[:, m0 : m0 + w]
                        )
                        for k, c, o, wk_ in ((k0, 0, o0, w0), (k1, c1, o1, w1)):
                            g = b * 16 + k
                            nc.tensor.matmul(
                                acc[:, o:512],
                                v_seq[:, g * 128 : (g + 1) * 128],
                                pT[:, c : c + wk_],
                                start=first_mm[0],
                                stop=(kind == "d1" and k == k1),
                            )
                            first_mm[0] = False
                        # fold k1's P into k0's columns (same q range), then
                        # one denominator matmul for the pair.
                        nc.vector.tensor_add(
                            pT[:, o1 - o0 : w0], pT[:, o1 - o0 : w0],
                            pT[:, c1 : c1 + w1],
                        )
                        den_mm(pT[:, :w0], o0, last=(kind == "d1"))

                # den rows are identical (all-ones stationary) == denominator
                # already broadcast across partitions.
                rb_sb = sb.tile([128, 512], F32, tag="rb_sb")
                nc.vector.reciprocal_approx_fast(rb_sb[:], den[:])
                ao = sb.tile([128, 512], BF, tag="ao", bufs=3)
                nc.vector.tensor_mul(ao[:], acc[:], rb_sb[:])
                sc = b * 4 + qc
                if h < 2:
                    dst = a2a_in[0][sc * 256 + h * 128 : sc * 256 + (h + 1) * 128, :]
                else:
                    dst = a2a_in[h - 1][sc * 128 : (sc + 1) * 128, :]
                nc.sync.dma_start(dst, ao[:])

            with tc.tile_pool(name="wop", bufs=1) as wop, \
                 tc.tile_pool(name="agp", bufs=1) as agp, \
                 tc.tile_pool(name="prt", bufs=1) as prt:
                wo_tiles = []

                def load_wo(t):
                    wt = wop.tile([128, 2048], BF, tag="wo_t", bufs=16)
                    nc.gpsimd.dma_start(wt[:], wo_e[:, t * 2048 : (t + 1) * 2048])
                    wo_tiles.append(wt)

                def a2a(i):
                    nc.gpsimd.collective_compute(
                        "AllToAll",
                        mybir.AluOpType.bypass,
                        replica_groups=[list(range(N_CORES))],
                        ins=[a2a_in[i].opt()],
                        outs=[a2a_out[i].opt()],
                    )

                agt0 = agp.tile([128, 16, 512], BF, tag="ag0")
                agt1 = agp.tile([128, 16, 512], BF, tag="ag1")
                parts = []

                def pass0_chain(hid_t):
                    o_ps = ps.tile([128, 512], F32, tag="ad", bufs=4)
                    for ft in range(16):
                        nc.tensor.matmul(
                            o_ps[:],
                            wo_tiles[hid_t][:, ft * 128 : (ft + 1) * 128],
                            agt0[:, ft, :],
                            start=(ft == 0),
                            stop=(ft == 15),
                        )
                    part = prt.tile([128, 512], BF, tag=f"part{hid_t}")
                    nc.scalar.activation(part[:], o_ps[:], AF.Copy)
                    parts.append(part)

                with nc.named_scope("attn"):
                    # pass-0 o_proj weights prefetch on the idle gpsimd
                    # queue; they land during attention so the PE never
                    # waits at the attention->o_proj boundary.
                    for t in range(16):
                        load_wo(t)
                    for h in (0, 1):
                        for b in range(B):
                            for qc in range(4):
                                attn_head(h, b, qc)
                    a2a(0)
                    # single big gather: gated on collective 0 only
                    nc.gpsimd.dma_start(
                        agt0[:],
                        a2a_out[0][:].rearrange("(ft p) s -> p ft s", p=128),
                    )
                    for b in range(B):
                        for qc in range(4):
                            attn_head(2, b, qc)
                    a2a(1)
                    # head-2 feature blocks land in agt1's first half
                    nc.gpsimd.dma_start(
                        agt1[:, 0:8, :],
                        a2a_out[1][:].rearrange("(ft p) s -> p ft s", p=128),
                    )
                    for b in range(B):
                        for qc in range(4):
                            attn_head(3, b, qc)
                    a2a(2)

                # ---- phase 4: o_proj pass 1 (features from head-2/3 A2As).
                with nc.named_scope("oproj"):
                    for t in range(16, 32):  # pass-1 weights stream
                        load_wo(t)
                    nc.gpsimd.dma_start(
                        agt1[:, 8:16, :],
                        a2a_out[2][:].rearrange("(ft p) s -> p ft s", p=128),
                    )
                    # pass 0 runs here, overlapping the head-3 collective
                    # and the agt1 load it gates.
                    for hid_t in range(16):
                        pass0_chain(hid_t)
                    for hid_t in range(NKT):  # 16 tiles of 128 hidden cols
                        wo_t = wo_tiles[16 + hid_t]
                        o_ps = ps.tile([128, 512], F32, tag="ad", bufs=4)
                        for ft in range(16):
                            nc.tensor.matmul(
                                o_ps[:],
                                wo_t[:, ft * 128 : (ft + 1) * 128],
                                agt1[:, ft, :],
                                start=(ft == 0),
                                stop=(ft == 15),
                            )
                        ob = sb.tile([128, 512], F32, tag="ob", bufs=3)
                        nc.vector.tensor_add(ob[:], o_ps[:], parts[hid_t][:])
                        nc.sync.dma_start(
                            outT_e[hid_t * 128 : (hid_t + 1) * 128, :], ob[:]
                        )

    nc.compile()
    return nc


def _prep(hidden_states, sin_table, cos_table, Wq, Wk, Wv, Wo):
    bf = ml_dtypes.bfloat16
    flat = np.asarray(hidden_states, np.float32).reshape(SG, HID)
    hT = np.ascontiguousarray(flat.T).astype(bf)

    cosT = np.asarray(cos_table, np.float32)[:, :64].T  # [64, S]
    sinT = np.asarray(sin_table, np.float32)[:, :64].T
    c_dup = np.tile(np.concatenate([cosT, cosT], 0), (1, B)).astype(bf)
    # sign-folded: rotate-half becomes a plain partition swap
    s_dup = np.tile(np.concatenate([-sinT, sinT], 0), (1, B)).astype(bf)

    kk = np.arange(128)[:, None]
    tril = lambda w: (kk <= np.arange(w)[None, :]).astype(np.float32)
    dmask = np.concatenate(
        [tril(512), tril(384), tril(256), tril(128)], axis=1
    ).astype(bf)

    scale = np.float32(1.0 / np.sqrt(D))
    Wq = np.asarray(Wq, np.float32) * scale
    Wk = np.asarray(Wk, np.float32)
    Wv = np.asarray(Wv, np.float32)
    Wo = np.asarray(Wo, np.float32)

    # Permute Wo rows into the order o_proj consumes the A2A output blocks:
    # a2a1 blocks: (r, h in {0,1}); a2a2 blocks: (r, h in {2,3}). Then lay
    # out as [p, (half, hid_t, ft, c)] so each weight tile is one contiguous
    # [128, 2048] DMA.
    Wo_b = Wo.reshape(H, D, HID)
    order = (
        [4 * r + h for r in range(8) for h in (0, 1)]
        + [4 * r + 2 for r in range(8)]
        + [4 * r + 3 for r in range(8)]
    )
    Wo_perm = Wo_b[order].reshape(H * D, HID)
    W5 = Wo_perm.reshape(2, 16, 128, 16, 128)       # [half, ft, p, hid_t, c]
    woL = np.ascontiguousarray(
        W5.transpose(2, 0, 3, 1, 4).reshape(128, 2 * 16 * 16 * 128)
    ).astype(bf)

    in_maps = []
    for c in range(N_CORES):
        in_maps.append(
            {
                "hT": hT,
                "wq": np.ascontiguousarray(Wq[:, c * 512 : (c + 1) * 512]).astype(bf),
                "wk": np.ascontiguousarray(Wk[:, c * D : (c + 1) * D]).astype(bf),
                "wv": np.ascontiguousarray(Wv[:, c * D : (c + 1) * D]).astype(bf),
                "wo": woL,
                "c_dup": c_dup,
                "s_dup": s_dup,
                "dmask": dmask,
            }
        )
    return in_maps


def kernel(**inputs) -> np.ndarray:
    global LAST_EXEC_NS
    if "nc" not in _CACHE:
        _CACHE["nc"] = _build()
    nc = _CACHE["nc"]

    in_maps = _prep(**inputs)
    res = run_bass_kernel_spmd(
        nc,
        in_maps,
        core_ids=list(range(N_CORES)),
        trace=bool(os.environ.get("BASS_TRACE")),
    )
    LAST_EXEC_NS = res.exec_time_ns

    outT = np.concatenate(
        [np.asarray(res.results[c]["outT"], np.float32) for c in range(N_CORES)],
        axis=1,
    )  # [HID, SG]
    return np.ascontiguousarray(outT.T).reshape(B, S, HID)
